# revision 29
# baseline (speedup 1.0000x reference)
"""Multi-head attention (RoPE, causal) Trainium2 Bass kernel, 8-way sharded.

Sharding: tensor-parallel over heads x data-parallel over batch.
  core c (0..7): batch b = c // 4, head group hg = c % 4 -> heads [4*hg, 4*hg+4).
Each core computes its 4 heads' QKV projection, RoPE, causal attention, and a
partial output projection (its 512 columns of the E-dim contraction).  The host
sums the 4 partials per batch and transposes back.

Device-side layouts are transposed ([feature, seq]) so matmuls feed the PE
array directly.  All matmul operands are bf16 (fp32 PSUM accumulation), which
keeps every tensor SBUF-resident.  Softmax skips the max-subtraction (logits
are O(+-10) so fp32 exp cannot overflow) and stays in the [key, query]
orientation; the denominator is accumulated on the Vector engine (GPSIMD in
the final block, where DVE is the limiter) and partition-reduced with a
ones-matmul.

The whole kernel is one fused software-pipelined loop over 512-row q-blocks:
the QKV chains for block sb+1 and the output projection for block sb-1 are
emitted interleaved ("fillers") into block sb's attention tiles, so the
in-order Tensor queue always has independent matmuls to run while the
Activation engine works through the exp() pipeline.

v2 changes (trace-driven):
 - All inputs host-prepacked into contiguous-per-tile DRAM layouts and loaded
   with ~20 large DMAs instead of ~210 small ones (a DMA trigger costs ~615ns
   serially on the issuing engine; the old kernel's startup was trigger-rate
   bound for its first ~30us, holding the PE clock cold at 1.2GHz).
 - Startup triggers fan out across sync + scalar + gpsimd queues.
 - x blocks are prefetched one attention-window early, so filler QKV chains
   never wait on DMA.
 - Off-diagonal score tiles are exp'd in [128,1024] pairs (halves the ACT
   fixed cost where the Activation engine is the pipeline limiter).
 - Output stores staged into [128,1024] tiles (2 m-tiles per DMA).
 - Outproj PSUM drains alternate ACT/DVE in the late windows.
"""

import sys

sys.path.insert(0, "/opt/trn_rl_repo")

import numpy as np
import ml_dtypes

import concourse.bass as bass  # noqa: F401
import concourse.tile as tile
from concourse import bacc, mybir
from concourse import bass_utils

# bass_utils' trace path imports antenv.axon_hooks, which may be absent from
# this image; register a no-op hook module so an externally-set BASS_TRACE
# degrades to "no profile" instead of crashing the run.
try:
    import antenv.axon_hooks  # noqa: F401
except ImportError:
    import types

    _hooks = types.ModuleType("antenv.axon_hooks")
    _hooks.get_axon_ntff_profile_hook = lambda: None
    _hooks.set_axon_ntff_profile_hook = lambda h: None
    sys.modules["antenv.axon_hooks"] = _hooks
    try:
        import antenv

        antenv.axon_hooks = _hooks
    except ImportError:
        pass

# Problem shape (hardcoded per contract).
B = 2
S = 2048
E = 2048
H = 16
D = 128
N_CORES = 8
GPB = N_CORES // B  # head groups per batch = 4
HPC = H // GPB  # heads per core = 4
DPC = HPC * D  # feature cols per core = 512
SBLK = 512
NSBLK = S // SBLK  # 4
NECH = E // 128  # 16 contraction chunks
SM_SCALE = float(D) ** -0.5

F32 = mybir.dt.float32
BF16 = mybir.dt.bfloat16

_CACHE = {}
_RUN_KWARGS = {}


def _build_nc():
    nc = bacc.Bacc(
        "TRN2",
        target_bir_lowering=False,
        debug=False,
        enable_asserts=False,
        num_devices=N_CORES,
    )
    # host-prepacked inputs; each leading-index slice is one contiguous DMA
    xq_d = nc.dram_tensor("xq", [NSBLK, 4, 128, 2048], BF16, kind="ExternalInput").ap()
    wqm_d = nc.dram_tensor("wqm", [HPC, 128, 2048], BF16, kind="ExternalInput").ap()
    wkm_d = nc.dram_tensor("wkm", [HPC, 128, 2048], BF16, kind="ExternalInput").ap()
    wvh_d = nc.dram_tensor("wvh", [2, 128, 4096], BF16, kind="ExternalInput").ap()
    woh_d = nc.dram_tensor("woh", [2, 128, 4096], BF16, kind="ExternalInput").ap()
    # cos | sin(first half negated) packed; bmask separate (needed later)
    tab_d = nc.dram_tensor("tabs", [128, 4096], BF16, kind="ExternalInput").ap()
    bm_d = nc.dram_tensor("bmask", [128, 2048], BF16, kind="ExternalInput").ap()
    # output: [p][m][s] -> host reads as E = m*128+p
    outT = nc.dram_tensor("outT", [128, 16, 2048], BF16, kind="ExternalOutput").ap()

    with tile.TileContext(nc) as tc, nc.allow_low_precision(reason="bf16 matmuls"):
        with (
            tc.tile_pool(name="persist", bufs=1) as persist,
            tc.tile_pool(name="xq", bufs=8) as xq_pool,
            tc.tile_pool(name="rt1", bufs=2) as t1_pool,
            tc.tile_pool(name="rt2", bufs=2) as t2_pool,
            tc.tile_pool(name="ex", bufs=5) as ex_pool,
            tc.tile_pool(name="acc", bufs=2) as acc_pool,
            tc.tile_pool(name="rcpr", bufs=2) as rcpr_pool,
            tc.tile_pool(name="dbc", bufs=2) as dbc_pool,
            tc.tile_pool(name="st", bufs=2) as stage_pool,
            tc.tile_pool(name="pmm", bufs=2, space="PSUM") as pmm_pool,
            tc.tile_pool(name="psc", bufs=2, space="PSUM") as psc_pool,
            tc.tile_pool(name="pctx", bufs=2, space="PSUM") as pctx_pool,
        ):
            ones_bf = persist.tile([128, 1], BF16, tag="ones", name="ones_bf")
            nc.vector.memset(ones_bf[:], 1.0)
            warm_sb = persist.tile([128, 256], BF16, tag="warm", name="warm_sb")
            nc.vector.memset(warm_sb[:], 0.0)
            tab_sb = persist.tile([128, 4096], BF16, tag="tab", name="tab_sb")
            bm_sb = persist.tile([128, 2048], BF16, tag="bm", name="bm_sb")
            cos_sb = tab_sb[:, 0:2048]
            sin_sb = tab_sb[:, 2048:4096]
            wq_t = [
                persist.tile([128, 2048], BF16, tag=f"wq{m}", name=f"wq{m}")
                for m in range(HPC)
            ]
            wk_t = [
                persist.tile([128, 2048], BF16, tag=f"wk{m}", name=f"wk{m}")
                for m in range(HPC)
            ]
            wv_t = [
                persist.tile([128, 4096], BF16, tag=f"wv{i}", name=f"wv{i}")
                for i in range(2)
            ]
            wo_t = [
                persist.tile([128, 4096], BF16, tag=f"wo{i}", name=f"wo{i}")
                for i in range(2)
            ]
            q_sb = [
                persist.tile([128, S], BF16, tag=f"q{h}", name=f"q{h}")
                for h in range(HPC)
            ]
            k_sb = [
                persist.tile([128, S], BF16, tag=f"k{h}", name=f"k{h}")
                for h in range(HPC)
            ]
            v_sb = persist.tile([128, 16 * DPC], BF16, tag="v", name="v_sb")
            ctx16 = [
                [
                    persist.tile([128, SBLK], BF16, tag=f"c{h}_{t}", name=f"c{h}_{t}")
                    for t in range(NSBLK)
                ]
                for h in range(HPC)
            ]

            # ---- slice helpers over the packed layouts ----
            def wq_sl(m, e):  # lhsT [128 E-rows, 128 q-feature cols] for chunk e
                return wq_t[m][:, e * 128 : (e + 1) * 128]

            def wk_sl(m, e):
                return wk_t[m][:, e * 128 : (e + 1) * 128]

            def wv_sl(e):  # rhs [128 E-rows, 512 v-features] for chunk e
                return wv_t[e // 8][:, (e % 8) * 512 : (e % 8 + 1) * 512]

            def wo_sl(h, m):  # lhsT [128 dpc-rows of head h, 128 E-cols]
                base = (h % 2) * 2048
                return wo_t[h // 2][:, base + m * 128 : base + (m + 1) * 128]

            # xq tiles per block: 4 quarter tiles [128, 2048]; quarter qi holds
            # e-chunks 4qi..4qi+3 at col (e%4)*512
            xtiles = {}

            def load_xblock(sb, eng, quarters=None):
                ts = xtiles.setdefault(sb, [None] * 4)
                for qi in quarters if quarters is not None else range(4):
                    t = xq_pool.tile([128, 2048], BF16, tag="xq", name=f"x{sb}_{qi}")
                    eng.dma_start(t[:], xq_d[sb, qi])
                    ts[qi] = t

            def x_sl(sb, e):  # [128, 512] chunk e of block sb
                t = xtiles[sb][e // 4]
                return t[:, (e % 4) * 512 : (e % 4 + 1) * 512]

            # ---- startup DMAs: fan out across three trigger queues, in
            # consumption order; the startup is aggregate-HBM-bound so only
            # first-needed tensors (x0, wq/wk, rope tables) go early and the
            # rest (x1, wv, wo, bmask) queue behind them.  The very first
            # x-quarter and wq tile are split in halves so the first chain's
            # matmuls gate on 0.25MB instead of 0.5MB. ----
            x00 = xq_pool.tile([128, 2048], BF16, tag="xq", name="x0_0")
            nc.sync.dma_start(x00[:, 0:1024], xq_d[0, 0, :, 0:1024])
            nc.sync.dma_start(x00[:, 1024:2048], xq_d[0, 0, :, 1024:2048])
            xtiles.setdefault(0, [None] * 4)[0] = x00
            nc.scalar.dma_start(wq_t[0][:, 0:1024], wqm_d[0][:, 0:1024])
            nc.scalar.dma_start(wq_t[0][:, 1024:2048], wqm_d[0][:, 1024:2048])
            load_xblock(0, nc.gpsimd, quarters=(2,))
            load_xblock(0, nc.sync, quarters=(1,))
            nc.scalar.dma_start(wk_t[0][:], wkm_d[0])
            load_xblock(0, nc.gpsimd, quarters=(3,))
            nc.scalar.dma_start(wq_t[1][:], wqm_d[1])
            nc.sync.dma_start(wq_t[2][:], wqm_d[2])
            nc.scalar.dma_start(wk_t[1][:], wkm_d[1])
            nc.sync.dma_start(wk_t[2][:], wkm_d[2])
            nc.gpsimd.dma_start(tab_sb[:], tab_d[:, :])
            nc.sync.dma_start(wq_t[3][:], wqm_d[3])
            nc.sync.dma_start(wk_t[3][:], wkm_d[3])
            nc.scalar.dma_start(wv_t[1][:], wvh_d[1])
            nc.gpsimd.dma_start(wv_t[0][:], wvh_d[0])
            nc.scalar.dma_start(bm_sb[:], bm_d[:, :])
            load_xblock(1, nc.scalar)
            nc.gpsimd.dma_start(wo_t[0][:], woh_d[0])
            nc.gpsimd.dma_start(wo_t[1][:], woh_d[1])

            def rope_to(ps, dst, ssl):
                # dst[:, ssl] = ps*cos + rotate_half(ps)*sin  (sin pre-negated
                # in its first half by the host table)
                t1 = t1_pool.tile([128, SBLK], BF16, tag="t1", name="t1")
                nc.vector.tensor_mul(t1[:], ps[:], cos_sb[:, ssl])
                t2 = t2_pool.tile([128, SBLK], BF16, tag="t2", name="t2")
                nc.vector.tensor_mul(t2[0:64, :], ps[64:128, :], sin_sb[0:64, ssl])
                nc.vector.tensor_mul(t2[64:128, :], ps[0:64, :], sin_sb[64:128, ssl])
                nc.vector.tensor_add(dst[:, ssl], t1[:], t2[:])

            def gen_qk(sb):
                # yields once per Tensor matmul (128 total)
                ssl = slice(sb * SBLK, (sb + 1) * SBLK)
                for m in range(HPC):
                    for w_sl, dst in ((wq_sl, q_sb), (wk_sl, k_sb)):
                        ps = pmm_pool.tile([128, SBLK], F32, tag="mm", name="psqk")
                        for e in range(NECH):
                            nc.tensor.matmul(
                                ps[:], w_sl(m, e), x_sl(sb, e),
                                start=(e == 0), stop=(e == NECH - 1),
                                skip_group_check=True,
                            )
                            if e == NECH - 1:
                                rope_to(ps, dst[m], ssl)
                            yield

            def gen_v(sb):
                # yields once per Tensor matmul (64 total)
                for sm in range(SBLK // 128):
                    st = sb * (SBLK // 128) + sm
                    ps = pmm_pool.tile([128, DPC], F32, tag="mm", name="psv")
                    for e in range(NECH):
                        nc.tensor.matmul(
                            ps[:], x_sl(sb, e)[:, sm * 128 : (sm + 1) * 128], wv_sl(e),
                            start=(e == 0), stop=(e == NECH - 1),
                            skip_group_check=True,
                        )
                        if e == NECH - 1:
                            nc.scalar.copy(v_sb[:, st * DPC : (st + 1) * DPC], ps[:])
                        yield

            def gen_qkv(sb):
                yield from gen_qk(sb)
                yield from gen_v(sb)

            def gen_outproj(sb, drain="act", fan_tail=False):
                # yields once per Tensor matmul (64 total); drain: which
                # engine(s) copy PSUM->SBUF ('act' | 'split'); stores are
                # staged in [128,1024] tiles, one DMA per 2 m-tiles.
                # fan_tail (epilogue): the last 4 m-tiles store per-m on
                # rotating queues so the final store transfer is 128KB on an
                # otherwise-empty queue instead of 256KB behind everything.
                ssl = slice(sb * SBLK, (sb + 1) * SBLK)
                tail_eng = [nc.gpsimd, nc.sync, nc.gpsimd, nc.sync]
                stage = None
                for m in range(E // 128):
                    gsz = 1 if (fan_tail and m >= 12) else 2
                    if gsz == 2 and m % 2 == 0 or gsz == 1:
                        stage = stage_pool.tile([128, 1024], BF16, tag="st", name="st")
                    po = pmm_pool.tile([128, SBLK], F32, tag="mm", name="po")
                    for h in range(HPC):
                        nc.tensor.matmul(
                            po[:], wo_sl(h, m), ctx16[h][sb][:],
                            start=(h == 0), stop=(h == HPC - 1),
                            skip_group_check=True,
                        )
                        if h == HPC - 1:
                            si = (m % 2) if gsz == 2 else 0
                            dst = stage[:, si * 512 : (si + 1) * 512]
                            if drain == "split" and m % 2:
                                nc.vector.tensor_scalar_mul(dst, po[:], 1.0)
                            else:
                                nc.scalar.copy(dst, po[:])
                            if gsz == 1:
                                tail_eng[m % 4].dma_start(
                                    outT[:, m : m + 1, ssl], stage[:, 0:512]
                                )
                            elif m % 2 == 1:
                                nc.sync.dma_start(
                                    outT[:, m - 1 : m + 1, ssl], stage[:]
                                )
                        yield

            # per-(sb,h) attention state carried across window boundaries for
            # the tail-smoothing split of the last block
            astate = {}

            def attn_head(sb, h, jt_lo, jt_hi, pump):
                njt = 4 * (sb + 1)
                # The very last head's softmax tail gates the epilogue behind
                # the deep end-of-kernel DVE queue; for it alone, accumulate
                # the denominator with per-tile ones-matmuls on the (then
                # slack) Tensor engine so the tail is ~2us.
                den_on_pe = sb == NSBLK - 1 and h == HPC - 1
                key = (sb, h)
                if jt_lo == 0:
                    acc = den_ps = None
                    if not den_on_pe:
                        acc = acc_pool.tile([128, SBLK], BF16, tag="acc", name="acc")
                    ctx_ps = pctx_pool.tile([128, SBLK], F32, tag="ctx", name="ctxps")
                    if den_on_pe:
                        den_ps = pctx_pool.tile([1, SBLK], F32, tag="ctx", name="denps2")
                    astate[key] = (acc, ctx_ps, den_ps)
                else:
                    acc, ctx_ps, den_ps = astate[key]

                def emit_ctx(work):
                    jt, lo, ex = work
                    if den_on_pe:
                        nc.tensor.matmul(
                            den_ps[0:1, lo:SBLK],
                            ones_bf[:],
                            ex[:, lo:SBLK],
                            start=(jt == 0),
                            stop=(jt == njt - 1),
                            skip_group_check=True,
                        )
                    nc.tensor.matmul(
                        ctx_ps[:, lo:SBLK],
                        v_sb[:, jt * DPC + h * 128 : jt * DPC + (h + 1) * 128],
                        ex[:, lo:SBLK],
                        start=(jt == 0),
                        stop=(jt == njt - 1),
                        skip_group_check=True,
                    )

                inflight = []
                # j-tiles processed in pairs sharing a [128,1024] PSUM tile;
                # off-diagonal pairs get a single wide exp
                for pj in range(jt_lo // 2, jt_hi // 2):
                    jta = 2 * pj
                    scp = psc_pool.tile([128, 2 * SBLK], F32, name="scp")
                    halves = []
                    for ji in range(2):
                        jt = jta + ji
                        o = jt - 4 * sb
                        lo = max(o, 0) * 128
                        hof = ji * SBLK
                        nc.tensor.matmul(
                            scp[:, hof + lo : hof + SBLK],
                            k_sb[h][:, jt * 128 : (jt + 1) * 128],
                            q_sb[h][:, sb * SBLK + lo : (sb + 1) * SBLK],
                            start=True,
                            stop=True,
                            skip_group_check=True,
                        )
                        halves.append((jt, o, lo, hof))
                    ex = ex_pool.tile([128, 2 * SBLK], BF16, tag="ex", name="ex")
                    if halves[0][1] < 0 and halves[1][1] < 0:
                        # off-diagonal pair: one wide exp
                        nc.scalar.activation(
                            ex[:],
                            scp[:],
                            mybir.ActivationFunctionType.Exp,
                            scale=SM_SCALE,
                        )
                    else:
                        for jt, o, lo, hof in halves:
                            nc.scalar.activation(
                                ex[:, hof + lo : hof + SBLK],
                                scp[:, hof + lo : hof + SBLK],
                                mybir.ActivationFunctionType.Exp,
                                scale=SM_SCALE,
                            )
                    for jt, o, lo, hof in halves:
                        exh = ex[:, hof : hof + SBLK]
                        if o >= 0:
                            nc.vector.tensor_mul(
                                exh[:, lo:SBLK],
                                exh[:, lo:SBLK],
                                bm_sb[:, o * SBLK + lo : (o + 1) * SBLK],
                            )
                        if not den_on_pe:
                            if jt == 0:
                                nc.vector.tensor_scalar_mul(acc[:], exh[:], 1.0)
                            else:
                                nc.vector.tensor_add(
                                    acc[:, lo:SBLK], acc[:, lo:SBLK],
                                    exh[:, lo:SBLK],
                                )
                        inflight.append((jt, lo, exh))
                        if len(inflight) > 6:
                            emit_ctx(inflight.pop(0))
                        pump()
                for work in inflight:
                    emit_ctx(work)
                if jt_hi < njt:
                    return
                if not den_on_pe:
                    # denominator: gpsimd cross-partition reduce of the bf16
                    # acc -> [1,512] reciprocal -> gpsimd bcast (keeps the
                    # partition sum off the Tensor engine's critical path)
                    den_ps = rcpr_pool.tile([1, SBLK], F32, tag="rr", name="denr")
                    nc.gpsimd.tensor_reduce(
                        den_ps[:], acc[:],
                        axis=mybir.AxisListType.C, op=mybir.AluOpType.add,
                    )
                rcp_row = rcpr_pool.tile([1, SBLK], F32, tag="rr", name="rr")
                nc.vector.reciprocal_approx_fast(out=rcp_row[:], in_=den_ps[:])
                dbc = dbc_pool.tile([128, SBLK], F32, tag="db", name="db")
                nc.gpsimd.partition_broadcast(dbc[:], rcp_row[:])
                nc.vector.tensor_mul(ctx16[h][sb][:], ctx_ps[:], dbc[:])

            def attn(segments, filler_iter, total_steps):
                n_tiles = sum(hi - lo for _, _, lo, hi in segments)
                tile_i = 0
                drawn = 0

                def pump():
                    nonlocal tile_i, drawn
                    tile_i += 1
                    want = (total_steps * tile_i) // n_tiles
                    if want > drawn:
                        for _ in range(want - drawn):
                            next(filler_iter, None)
                        drawn = want

                for sb, h, lo, hi in segments:
                    attn_head(sb, h, lo, hi, pump)
                for _ in iter(lambda: next(filler_iter, StopIteration), StopIteration):
                    pass

            # ---- prologue: q/k projection for block 0, no interleave; the
            # v projection of block 0 runs as window-0 filler so the PE stays
            # dense through the attention pipeline fill ----
            for _ in gen_qk(0):
                pass
            # ---- fused attention loop; the last block's heads 0-1 first-half
            # j-tiles run at the end of window 2 so the exp/acc load of the
            # (otherwise ACT/DVE-bound) final block spreads over two windows
            SPLIT = 8  # first-half j-tiles of sb=3 h=0,1 handled in window 2
            for sb in range(NSBLK):
                # prefetch x for block sb+2 (consumed by fillers next window)
                if sb + 2 < NSBLK:
                    load_xblock(sb + 2, nc.sync)
                seq = []
                total = 0
                qg = None
                if sb + 1 < NSBLK:
                    qg = gen_qkv(sb + 1)
                    total += 2 * HPC * NECH + (SBLK // 128) * NECH
                if sb == 0:
                    vg = gen_v(0)
                    total += (SBLK // 128) * NECH
                    seq = [(vg, 1 << 30), (qg, 1 << 30)]
                elif sb >= 1:
                    og = gen_outproj(sb - 1, drain="split" if sb >= 3 else "act")
                    total += (E // 128) * HPC
                    if qg is not None:
                        seq = [(qg, 16), (og, 1 << 30), (qg, 1 << 30)]
                    else:
                        seq = [(og, 1 << 30)]

                def filler_iter_fn(entries):
                    for g, cap in entries:
                        n = 0
                        while n < cap:
                            try:
                                next(g)
                            except StopIteration:
                                break
                            n += 1
                            yield

                njt = 4 * (sb + 1)
                segments = [(sb, h, 0, njt) for h in range(HPC)]
                if sb == NSBLK - 2:
                    segments += [(NSBLK - 1, 0, 0, SPLIT), (NSBLK - 1, 1, 0, SPLIT)]
                elif sb == NSBLK - 1:
                    segments = [
                        (sb, 0, SPLIT, njt),
                        (sb, 1, SPLIT, njt),
                        (sb, 2, 0, njt),
                        (sb, 3, 0, njt),
                    ]
                attn(segments, filler_iter_fn(seq), total)
            # ---- epilogue: output projection for the last block; exp is
            # done by now so ACT is free to take every drain, keeping DVE
            # clear for the last heads' normalization chain.  A few dummy
            # matmuls bridge the PE idle gap while that chain completes so
            # the clock gate stays at 8/8 for the final projection ----
            warm_ps = psc_pool.tile([1, 256], F32, name="scp")
            for _ in range(10):
                nc.tensor.matmul(
                    warm_ps[:], ones_bf[:], warm_sb[:],
                    start=True, stop=True, skip_group_check=True,
                )
            for _ in gen_outproj(NSBLK - 1, drain="act", fan_tail=True):
                pass

    nc.compile()
    return nc


def _rope_tables():
    inv_freq = 1.0 / (10000.0 ** (np.arange(0, D, 2, dtype=np.float64) / D))
    t = np.arange(S, dtype=np.float64)
    freqs = np.outer(t, inv_freq)  # (S, D/2)
    emb = np.concatenate([freqs, freqs], axis=-1)  # (S, D)
    cosT = np.cos(emb).T.astype(np.float32).copy()  # (D, S)
    sinT = np.sin(emb).T.astype(np.float32)
    sinTs = sinT.copy()
    sinTs[: D // 2] = -sinT[: D // 2]
    return cosT, np.ascontiguousarray(sinTs)


def _binmask():
    r = np.arange(128)[:, None]
    c = np.arange(SBLK)[None, :]
    blocks = [(r + o * 128 <= c).astype(np.float32) for o in range(4)]
    return np.ascontiguousarray(np.concatenate(blocks, axis=1))


def _bf16(a):
    return np.ascontiguousarray(np.asarray(a, dtype=ml_dtypes.bfloat16))


def _numpy_fallback(x, mask, wqkv, bqkv, wout, bout):
    qkv = x @ wqkv.T + bqkv
    q, k, v = np.split(qkv, 3, axis=-1)
    q = q.reshape(B, S, H, D).transpose(0, 2, 1, 3)
    k = k.reshape(B, S, H, D).transpose(0, 2, 1, 3)
    v = v.reshape(B, S, H, D).transpose(0, 2, 1, 3)
    inv_freq = 1.0 / (10000.0 ** (np.arange(0, D, 2, dtype=np.float32) / D))
    t = np.arange(S, dtype=np.float32)
    freqs = np.outer(t, inv_freq)
    emb = np.concatenate([freqs, freqs], axis=-1)
    cos, sin = np.cos(emb), np.sin(emb)

    def rot(a):
        a1, a2 = np.split(a, 2, axis=-1)
        return np.concatenate([-a2, a1], axis=-1)

    q = q * cos + rot(q) * sin
    k = k * cos + rot(k) * sin
    scores = np.einsum("bhqd,bhkd->bhqk", q, k) * SM_SCALE
    scores = np.where(mask, -np.inf, scores)
    scores = scores - scores.max(axis=-1, keepdims=True)
    w = np.exp(scores)
    w = w / w.sum(axis=-1, keepdims=True)
    ctx = np.einsum("bhqk,bhkd->bhqd", w, v)
    ctx = ctx.transpose(0, 2, 1, 3).reshape(B, S, E)
    return (ctx @ wout.T + bout).astype(np.float32)


def _pack_inputs(x, wqkv, wout):
    """Per-core host packing into the contiguous-DMA DRAM layouts."""
    cosT, sinTs = _rope_tables()  # (D, S)
    tabs = _bf16(np.concatenate([cosT, sinTs], axis=1))  # [128, 4096]
    bm = _bf16(_binmask())  # [128, 2048]

    in_maps = []
    for c in range(N_CORES):
        b, hg = divmod(c, GPB)
        cols = slice(hg * DPC, (hg + 1) * DPC)
        wq = wqkv[0 * E : 1 * E, :][cols, :]  # (512, E): [feat, E]
        wk = wqkv[1 * E : 2 * E, :][cols, :]
        wv = wqkv[2 * E : 3 * E, :][cols, :]
        wo = wout[:, cols]  # (E, 512)

        # x: [sb][qi][p][(e%4)*512+s] = x[b, sb*512+s, (4qi+e')*128+p]
        xb = np.asarray(x[b], dtype=ml_dtypes.bfloat16)  # (S, E)
        xr = xb.reshape(NSBLK, SBLK, NECH, 128)  # [sb, s, e, p]
        xqp = np.ascontiguousarray(
            xr.transpose(0, 2, 3, 1).reshape(NSBLK, 4, 4, 128, SBLK)
            .transpose(0, 1, 3, 2, 4).reshape(NSBLK, 4, 128, 2048)
        )
        # wqm: [m][p][e*128+c] = wq[m*128+c, e*128+p]
        def pack_w(w):  # w: (512, E) [feat, E]
            wr = w.reshape(HPC, 128, NECH, 128)  # [m, c, e, p]
            return np.ascontiguousarray(
                wr.transpose(0, 3, 2, 1).reshape(HPC, 128, 2048).astype(
                    ml_dtypes.bfloat16
                )
            )

        wqm = pack_w(wq)
        wkm = pack_w(wk)
        # wvh: [hh][p][(e%8)*512+f] = wv[f, e*128+p]
        wvr = wv.reshape(DPC, NECH, 128)  # [f, e, p]
        wvh = np.ascontiguousarray(
            wvr.transpose(1, 2, 0).reshape(2, 8, 128, DPC)
            .transpose(0, 2, 1, 3).reshape(2, 128, 4096).astype(ml_dtypes.bfloat16)
        )
        # woh: [hh][p][(h%2)*2048+cc] = wo[cc, h*128+p]
        wor = wo.reshape(E, HPC, 128)  # [cc, h, p]
        woh = np.ascontiguousarray(
            wor.transpose(1, 2, 0).reshape(2, 2, 128, E)
            .transpose(0, 2, 1, 3).reshape(2, 128, 4096).astype(ml_dtypes.bfloat16)
        )
        in_maps.append(
            {
                "xq": xqp,
                "wqm": wqm,
                "wkm": wkm,
                "wvh": wvh,
                "woh": woh,
                "tabs": tabs,
                "bmask": bm,
            }
        )
    return in_maps


def kernel(x, mask, wqkv, bqkv, wout, bout, **_):
    x = np.ascontiguousarray(np.asarray(x), dtype=np.float32)
    wqkv = np.ascontiguousarray(np.asarray(wqkv), dtype=np.float32)
    bqkv = np.asarray(bqkv, dtype=np.float32)
    wout = np.ascontiguousarray(np.asarray(wout), dtype=np.float32)
    bout = np.asarray(bout, dtype=np.float32)
    mask = np.asarray(mask)

    causal = np.array_equal(mask, np.triu(np.ones((S, S), dtype=bool), k=1))
    if not causal or np.any(bqkv):
        return _numpy_fallback(x, mask, wqkv, bqkv, wout, bout)

    if "nc" not in _CACHE:
        _CACHE["nc"] = _build_nc()
    nc = _CACHE["nc"]

    in_maps = _pack_inputs(x, wqkv, wout)

    res = bass_utils.run_bass_kernel_spmd(
        nc, in_maps, core_ids=list(range(N_CORES)), **_RUN_KWARGS
    )
    _CACHE["last_results"] = res

    out = np.empty((B, S, E), dtype=np.float32)
    for b in range(B):
        acc = res.results[b * GPB]["outT"].astype(np.float32)
        for g in range(1, GPB):
            acc += res.results[b * GPB + g]["outT"].astype(np.float32)
        # outT [p, m, s] -> [E=m*128+p, S] -> transpose to (S, E)
        out[b] = acc.transpose(1, 0, 2).reshape(E, S).T
    out += bout
    return out


# revision 30
# speedup vs baseline: 4.2426x; 4.2426x over previous
"""Multi-head attention (RoPE, causal) Trainium2 Bass kernel, 8-way sharded.

Sharding: tensor-parallel over heads x data-parallel over batch.
  core c (0..7): batch b = c // 4, head group hg = c % 4 -> heads [4*hg, 4*hg+4).
Each core computes its 4 heads' QKV projection, RoPE, causal attention, and a
partial output projection (its 512 columns of the E-dim contraction).  The host
sums the 4 partials per batch and transposes back.

Device-side layouts are transposed ([feature, seq]) so matmuls feed the PE
array directly.  All matmul operands are bf16 (fp32 PSUM accumulation), which
keeps every tensor SBUF-resident.  Softmax skips the max-subtraction (logits
are O(+-10) so fp32 exp cannot overflow) and stays in the [key, query]
orientation; the denominator is accumulated on the Vector engine (GPSIMD in
the final block, where DVE is the limiter) and partition-reduced with a
ones-matmul.

The whole kernel is one fused software-pipelined loop over 512-row q-blocks:
the QKV chains for block sb+1 and the output projection for block sb-1 are
emitted interleaved ("fillers") into block sb's attention tiles, so the
in-order Tensor queue always has independent matmuls to run while the
Activation engine works through the exp() pipeline.

v2 changes (trace-driven):
 - All inputs host-prepacked into contiguous-per-tile DRAM layouts and loaded
   with ~20 large DMAs instead of ~210 small ones (a DMA trigger costs ~615ns
   serially on the issuing engine; the old kernel's startup was trigger-rate
   bound for its first ~30us, holding the PE clock cold at 1.2GHz).
 - Startup triggers fan out across sync + scalar + gpsimd queues.
 - x blocks are prefetched one attention-window early, so filler QKV chains
   never wait on DMA.
 - Off-diagonal score tiles are exp'd in [128,1024] pairs (halves the ACT
   fixed cost where the Activation engine is the pipeline limiter).
 - Output stores staged into [128,1024] tiles (2 m-tiles per DMA).
 - Outproj PSUM drains alternate ACT/DVE in the late windows.
"""

import sys

sys.path.insert(0, "/opt/trn_rl_repo")

import numpy as np
import ml_dtypes

import concourse.bass as bass  # noqa: F401
import concourse.tile as tile
from concourse import bacc, mybir
from concourse import bass_utils

# bass_utils' trace path imports antenv.axon_hooks, which may be absent from
# this image; register a no-op hook module so an externally-set BASS_TRACE
# degrades to "no profile" instead of crashing the run.
try:
    import antenv.axon_hooks  # noqa: F401
except ImportError:
    import types

    _hooks = types.ModuleType("antenv.axon_hooks")
    _hooks.get_axon_ntff_profile_hook = lambda: None
    _hooks.set_axon_ntff_profile_hook = lambda h: None
    sys.modules["antenv.axon_hooks"] = _hooks
    try:
        import antenv

        antenv.axon_hooks = _hooks
    except ImportError:
        pass

# Problem shape (hardcoded per contract).
B = 2
S = 2048
E = 2048
H = 16
D = 128
N_CORES = 8
GPB = N_CORES // B  # head groups per batch = 4
HPC = H // GPB  # heads per core = 4
DPC = HPC * D  # feature cols per core = 512
SBLK = 512
NSBLK = S // SBLK  # 4
NECH = E // 128  # 16 contraction chunks
SM_SCALE = float(D) ** -0.5

F32 = mybir.dt.float32
BF16 = mybir.dt.bfloat16

_CACHE = {}
_RUN_KWARGS = {}


def _build_nc():
    nc = bacc.Bacc(
        "TRN2",
        target_bir_lowering=False,
        debug=False,
        enable_asserts=False,
        num_devices=N_CORES,
    )
    # host-prepacked inputs; each leading-index slice is one contiguous DMA
    xq_d = nc.dram_tensor("xq", [NSBLK, 4, 128, 2048], BF16, kind="ExternalInput").ap()
    wqm_d = nc.dram_tensor("wqm", [HPC, 128, 2048], BF16, kind="ExternalInput").ap()
    wkm_d = nc.dram_tensor("wkm", [HPC, 128, 2048], BF16, kind="ExternalInput").ap()
    wvh_d = nc.dram_tensor("wvh", [2, 128, 4096], BF16, kind="ExternalInput").ap()
    woh_d = nc.dram_tensor("woh", [2, 128, 4096], BF16, kind="ExternalInput").ap()
    # cos | sin(first half negated) packed; bmask separate (needed later)
    tab_d = nc.dram_tensor("tabs", [128, 4096], BF16, kind="ExternalInput").ap()
    bm_d = nc.dram_tensor("bmask", [128, 2048], BF16, kind="ExternalInput").ap()
    # output: [p][m][s] -> host reads as E = m*128+p
    outT = nc.dram_tensor("outT", [128, 16, 2048], BF16, kind="ExternalOutput").ap()

    with tile.TileContext(nc) as tc, nc.allow_low_precision(reason="bf16 matmuls"):
        with (
            tc.tile_pool(name="persist", bufs=1) as persist,
            tc.tile_pool(name="xq", bufs=8) as xq_pool,
            tc.tile_pool(name="rt1", bufs=2) as t1_pool,
            tc.tile_pool(name="rt2", bufs=2) as t2_pool,
            tc.tile_pool(name="ex", bufs=5) as ex_pool,
            tc.tile_pool(name="acc", bufs=2) as acc_pool,
            tc.tile_pool(name="rcpr", bufs=2) as rcpr_pool,
            tc.tile_pool(name="dbc", bufs=2) as dbc_pool,
            tc.tile_pool(name="st", bufs=2) as stage_pool,
            tc.tile_pool(name="pmm", bufs=2, space="PSUM") as pmm_pool,
            tc.tile_pool(name="psc", bufs=2, space="PSUM") as psc_pool,
            tc.tile_pool(name="pctx", bufs=2, space="PSUM") as pctx_pool,
        ):
            ones_bf = persist.tile([128, 1], BF16, tag="ones", name="ones_bf")
            nc.vector.memset(ones_bf[:], 1.0)
            warm_sb = persist.tile([128, 256], BF16, tag="warm", name="warm_sb")
            nc.vector.memset(warm_sb[:], 0.0)
            tab_sb = persist.tile([128, 4096], BF16, tag="tab", name="tab_sb")
            bm_sb = persist.tile([128, 2048], BF16, tag="bm", name="bm_sb")
            cos_sb = tab_sb[:, 0:2048]
            sin_sb = tab_sb[:, 2048:4096]
            wq_t = [
                persist.tile([128, 2048], BF16, tag=f"wq{m}", name=f"wq{m}")
                for m in range(HPC)
            ]
            wk_t = [
                persist.tile([128, 2048], BF16, tag=f"wk{m}", name=f"wk{m}")
                for m in range(HPC)
            ]
            wv_t = [
                persist.tile([128, 4096], BF16, tag=f"wv{i}", name=f"wv{i}")
                for i in range(2)
            ]
            wo_t = [
                persist.tile([128, 4096], BF16, tag=f"wo{i}", name=f"wo{i}")
                for i in range(2)
            ]
            q_sb = [
                persist.tile([128, S], BF16, tag=f"q{h}", name=f"q{h}")
                for h in range(HPC)
            ]
            k_sb = [
                persist.tile([128, S], BF16, tag=f"k{h}", name=f"k{h}")
                for h in range(HPC)
            ]
            v_sb = persist.tile([128, 16 * DPC], BF16, tag="v", name="v_sb")
            ctx16 = [
                [
                    persist.tile([128, SBLK], BF16, tag=f"c{h}_{t}", name=f"c{h}_{t}")
                    for t in range(NSBLK)
                ]
                for h in range(HPC)
            ]

            # ---- slice helpers over the packed layouts ----
            def wq_sl(m, e):  # lhsT [128 E-rows, 128 q-feature cols] for chunk e
                return wq_t[m][:, e * 128 : (e + 1) * 128]

            def wk_sl(m, e):
                return wk_t[m][:, e * 128 : (e + 1) * 128]

            def wv_sl(e):  # rhs [128 E-rows, 512 v-features] for chunk e
                return wv_t[e // 8][:, (e % 8) * 512 : (e % 8 + 1) * 512]

            def wo_sl(h, m):  # lhsT [128 dpc-rows of head h, 128 E-cols]
                base = (h % 2) * 2048
                return wo_t[h // 2][:, base + m * 128 : base + (m + 1) * 128]

            # xq tiles per block: 4 quarter tiles [128, 2048]; quarter qi holds
            # e-chunks 4qi..4qi+3 at col (e%4)*512
            xtiles = {}

            def load_xblock(sb, eng, quarters=None):
                ts = xtiles.setdefault(sb, [None] * 4)
                for qi in quarters if quarters is not None else range(4):
                    t = xq_pool.tile([128, 2048], BF16, tag="xq", name=f"x{sb}_{qi}")
                    eng.dma_start(t[:], xq_d[sb, qi])
                    ts[qi] = t

            def x_sl(sb, e):  # [128, 512] chunk e of block sb
                t = xtiles[sb][e // 4]
                return t[:, (e % 4) * 512 : (e % 4 + 1) * 512]

            # ---- startup DMAs: fan out across three trigger queues, in
            # consumption order; the startup is aggregate-HBM-bound so only
            # first-needed tensors (x0, wq/wk, rope tables) go early and the
            # rest (x1, wv, wo, bmask) queue behind them.  The very first
            # x-quarter and wq tile are split in halves so the first chain's
            # matmuls gate on 0.25MB instead of 0.5MB. ----
            x00 = xq_pool.tile([128, 2048], BF16, tag="xq", name="x0_0")
            nc.sync.dma_start(x00[:, 0:1024], xq_d[0, 0, :, 0:1024])
            nc.sync.dma_start(x00[:, 1024:2048], xq_d[0, 0, :, 1024:2048])
            xtiles.setdefault(0, [None] * 4)[0] = x00
            nc.scalar.dma_start(wq_t[0][:, 0:1024], wqm_d[0][:, 0:1024])
            nc.scalar.dma_start(wq_t[0][:, 1024:2048], wqm_d[0][:, 1024:2048])
            load_xblock(0, nc.gpsimd, quarters=(2,))
            load_xblock(0, nc.sync, quarters=(1,))
            nc.scalar.dma_start(wk_t[0][:], wkm_d[0])
            load_xblock(0, nc.gpsimd, quarters=(3,))
            nc.scalar.dma_start(wq_t[1][:], wqm_d[1])
            nc.sync.dma_start(wq_t[2][:], wqm_d[2])
            nc.scalar.dma_start(wk_t[1][:], wkm_d[1])
            nc.sync.dma_start(wk_t[2][:], wkm_d[2])
            nc.gpsimd.dma_start(tab_sb[:], tab_d[:, :])
            nc.sync.dma_start(wq_t[3][:], wqm_d[3])
            nc.sync.dma_start(wk_t[3][:], wkm_d[3])
            nc.scalar.dma_start(wv_t[1][:], wvh_d[1])
            nc.gpsimd.dma_start(wv_t[0][:], wvh_d[0])
            nc.scalar.dma_start(bm_sb[:], bm_d[:, :])
            load_xblock(1, nc.scalar)
            nc.gpsimd.dma_start(wo_t[0][:], woh_d[0])
            nc.gpsimd.dma_start(wo_t[1][:], woh_d[1])

            def rope_to(ps, dst, ssl):
                # dst[:, ssl] = ps*cos + rotate_half(ps)*sin  (sin pre-negated
                # in its first half by the host table)
                t1 = t1_pool.tile([128, SBLK], BF16, tag="t1", name="t1")
                nc.vector.tensor_mul(t1[:], ps[:], cos_sb[:, ssl])
                t2 = t2_pool.tile([128, SBLK], BF16, tag="t2", name="t2")
                nc.vector.tensor_mul(t2[0:64, :], ps[64:128, :], sin_sb[0:64, ssl])
                nc.vector.tensor_mul(t2[64:128, :], ps[0:64, :], sin_sb[64:128, ssl])
                nc.vector.tensor_add(dst[:, ssl], t1[:], t2[:])

            def gen_qk(sb):
                # yields once per Tensor matmul (128 total)
                ssl = slice(sb * SBLK, (sb + 1) * SBLK)
                for m in range(HPC):
                    for w_sl, dst in ((wq_sl, q_sb), (wk_sl, k_sb)):
                        ps = pmm_pool.tile([128, SBLK], F32, tag="mm", name="psqk")
                        for e in range(NECH):
                            nc.tensor.matmul(
                                ps[:], w_sl(m, e), x_sl(sb, e),
                                start=(e == 0), stop=(e == NECH - 1),
                                skip_group_check=True,
                            )
                            if e == NECH - 1:
                                rope_to(ps, dst[m], ssl)
                            yield

            def gen_v(sb):
                # yields once per Tensor matmul (64 total)
                for sm in range(SBLK // 128):
                    st = sb * (SBLK // 128) + sm
                    ps = pmm_pool.tile([128, DPC], F32, tag="mm", name="psv")
                    for e in range(NECH):
                        nc.tensor.matmul(
                            ps[:], x_sl(sb, e)[:, sm * 128 : (sm + 1) * 128], wv_sl(e),
                            start=(e == 0), stop=(e == NECH - 1),
                            skip_group_check=True,
                        )
                        if e == NECH - 1:
                            nc.scalar.copy(v_sb[:, st * DPC : (st + 1) * DPC], ps[:])
                        yield

            def gen_qkv(sb):
                yield from gen_qk(sb)
                yield from gen_v(sb)

            def gen_outproj(sb, drain="act", fan_tail=False):
                # yields once per Tensor matmul (64 total); drain: which
                # engine(s) copy PSUM->SBUF ('act' | 'split'); stores are
                # staged in [128,1024] tiles, one DMA per 2 m-tiles.
                # fan_tail (epilogue): the last 4 m-tiles store per-m on
                # rotating queues so the final store transfer is 128KB on an
                # otherwise-empty queue instead of 256KB behind everything.
                ssl = slice(sb * SBLK, (sb + 1) * SBLK)
                tail_eng = [nc.gpsimd, nc.sync, nc.gpsimd, nc.sync]
                stage = None
                for m in range(E // 128):
                    gsz = 1 if (fan_tail and m >= 12) else 2
                    if gsz == 2 and m % 2 == 0 or gsz == 1:
                        stage = stage_pool.tile([128, 1024], BF16, tag="st", name="st")
                    po = pmm_pool.tile([128, SBLK], F32, tag="mm", name="po")
                    for h in range(HPC):
                        nc.tensor.matmul(
                            po[:], wo_sl(h, m), ctx16[h][sb][:],
                            start=(h == 0), stop=(h == HPC - 1),
                            skip_group_check=True,
                        )
                        if h == HPC - 1:
                            si = (m % 2) if gsz == 2 else 0
                            dst = stage[:, si * 512 : (si + 1) * 512]
                            if drain == "split" and m % 2:
                                nc.vector.tensor_scalar_mul(dst, po[:], 1.0)
                            else:
                                nc.scalar.copy(dst, po[:])
                            if gsz == 1:
                                tail_eng[m % 4].dma_start(
                                    outT[:, m : m + 1, ssl], stage[:, 0:512]
                                )
                            elif m % 2 == 1:
                                nc.sync.dma_start(
                                    outT[:, m - 1 : m + 1, ssl], stage[:]
                                )
                        yield

            # per-(sb,h) attention state carried across window boundaries for
            # the tail-smoothing split of the last block
            astate = {}

            def attn_head(sb, h, jt_lo, jt_hi, pump):
                njt = 4 * (sb + 1)
                # The very last head's softmax tail gates the epilogue behind
                # the deep end-of-kernel DVE queue; for it alone, accumulate
                # the denominator with per-tile ones-matmuls on the (then
                # slack) Tensor engine so the tail is ~2us.
                den_on_pe = sb == NSBLK - 1 and h == HPC - 1
                key = (sb, h)
                if jt_lo == 0:
                    acc = den_ps = None
                    if not den_on_pe:
                        acc = acc_pool.tile([128, SBLK], BF16, tag="acc", name="acc")
                    ctx_ps = pctx_pool.tile([128, SBLK], F32, tag="ctx", name="ctxps")
                    if den_on_pe:
                        den_ps = pctx_pool.tile([1, SBLK], F32, tag="ctx", name="denps2")
                    astate[key] = (acc, ctx_ps, den_ps)
                else:
                    acc, ctx_ps, den_ps = astate[key]

                def emit_ctx(work):
                    jt, lo, ex = work
                    if den_on_pe:
                        nc.tensor.matmul(
                            den_ps[0:1, lo:SBLK],
                            ones_bf[:],
                            ex[:, lo:SBLK],
                            start=(jt == 0),
                            stop=(jt == njt - 1),
                            skip_group_check=True,
                        )
                    nc.tensor.matmul(
                        ctx_ps[:, lo:SBLK],
                        v_sb[:, jt * DPC + h * 128 : jt * DPC + (h + 1) * 128],
                        ex[:, lo:SBLK],
                        start=(jt == 0),
                        stop=(jt == njt - 1),
                        skip_group_check=True,
                    )

                inflight = []
                # j-tiles processed in pairs sharing a [128,1024] PSUM tile;
                # off-diagonal pairs get a single wide exp
                for pj in range(jt_lo // 2, jt_hi // 2):
                    jta = 2 * pj
                    scp = psc_pool.tile([128, 2 * SBLK], F32, name="scp")
                    halves = []
                    for ji in range(2):
                        jt = jta + ji
                        o = jt - 4 * sb
                        lo = max(o, 0) * 128
                        hof = ji * SBLK
                        nc.tensor.matmul(
                            scp[:, hof + lo : hof + SBLK],
                            k_sb[h][:, jt * 128 : (jt + 1) * 128],
                            q_sb[h][:, sb * SBLK + lo : (sb + 1) * SBLK],
                            start=True,
                            stop=True,
                            skip_group_check=True,
                        )
                        halves.append((jt, o, lo, hof))
                    ex = ex_pool.tile([128, 2 * SBLK], BF16, tag="ex", name="ex")
                    if halves[0][1] < 0 and halves[1][1] < 0:
                        # off-diagonal pair: one wide exp
                        nc.scalar.activation(
                            ex[:],
                            scp[:],
                            mybir.ActivationFunctionType.Exp,
                            scale=SM_SCALE,
                        )
                    else:
                        for jt, o, lo, hof in halves:
                            nc.scalar.activation(
                                ex[:, hof + lo : hof + SBLK],
                                scp[:, hof + lo : hof + SBLK],
                                mybir.ActivationFunctionType.Exp,
                                scale=SM_SCALE,
                            )
                    for jt, o, lo, hof in halves:
                        exh = ex[:, hof : hof + SBLK]
                        if o >= 0:
                            nc.vector.tensor_mul(
                                exh[:, lo:SBLK],
                                exh[:, lo:SBLK],
                                bm_sb[:, o * SBLK + lo : (o + 1) * SBLK],
                            )
                        if not den_on_pe:
                            if jt == 0:
                                nc.vector.tensor_scalar_mul(acc[:], exh[:], 1.0)
                            else:
                                nc.vector.tensor_add(
                                    acc[:, lo:SBLK], acc[:, lo:SBLK],
                                    exh[:, lo:SBLK],
                                )
                        inflight.append((jt, lo, exh))
                        if len(inflight) > 6:
                            emit_ctx(inflight.pop(0))
                        pump()
                for work in inflight:
                    emit_ctx(work)
                if jt_hi < njt:
                    return
                if not den_on_pe:
                    # denominator: acc is already bf16 -> K=128 ones-matmul
                    # partition sum -> [1,512] reciprocal -> gpsimd bcast
                    den_ps = pmm_pool.tile([1, SBLK], F32, tag="mm", name="denps")
                    nc.tensor.matmul(
                        den_ps[:],
                        ones_bf[:],
                        acc[:],
                        start=True,
                        stop=True,
                        skip_group_check=True,
                    )
                rcp_row = rcpr_pool.tile([1, SBLK], F32, tag="rr", name="rr")
                nc.vector.reciprocal_approx_fast(out=rcp_row[:], in_=den_ps[:])
                dbc = dbc_pool.tile([128, SBLK], F32, tag="db", name="db")
                nc.gpsimd.partition_broadcast(dbc[:], rcp_row[:])
                nc.vector.tensor_mul(ctx16[h][sb][:], ctx_ps[:], dbc[:])

            def attn(segments, filler_iter, total_steps):
                n_tiles = sum(hi - lo for _, _, lo, hi in segments)
                tile_i = 0
                drawn = 0

                def pump():
                    nonlocal tile_i, drawn
                    tile_i += 1
                    want = (total_steps * tile_i) // n_tiles
                    if want > drawn:
                        for _ in range(want - drawn):
                            next(filler_iter, None)
                        drawn = want

                for sb, h, lo, hi in segments:
                    attn_head(sb, h, lo, hi, pump)
                for _ in iter(lambda: next(filler_iter, StopIteration), StopIteration):
                    pass

            # ---- prologue: q/k projection for block 0, no interleave; the
            # v projection of block 0 runs as window-0 filler so the PE stays
            # dense through the attention pipeline fill ----
            for _ in gen_qk(0):
                pass
            # ---- fused attention loop; the last block's heads 0-1 first-half
            # j-tiles run at the end of window 2 so the exp/acc load of the
            # (otherwise ACT/DVE-bound) final block spreads over two windows
            SPLIT = 8  # first-half j-tiles of sb=3 h=0,1 handled in window 2
            for sb in range(NSBLK):
                # prefetch x for block sb+2 (consumed by fillers next window)
                if sb + 2 < NSBLK:
                    load_xblock(sb + 2, nc.sync)
                seq = []
                total = 0
                qg = None
                if sb + 1 < NSBLK:
                    qg = gen_qkv(sb + 1)
                    total += 2 * HPC * NECH + (SBLK // 128) * NECH
                if sb == 0:
                    vg = gen_v(0)
                    total += (SBLK // 128) * NECH
                    seq = [(vg, 1 << 30), (qg, 1 << 30)]
                elif sb >= 1:
                    og = gen_outproj(sb - 1, drain="split" if sb >= 3 else "act")
                    total += (E // 128) * HPC
                    if qg is not None:
                        seq = [(qg, 16), (og, 1 << 30), (qg, 1 << 30)]
                    else:
                        seq = [(og, 1 << 30)]

                def filler_iter_fn(entries):
                    for g, cap in entries:
                        n = 0
                        while n < cap:
                            try:
                                next(g)
                            except StopIteration:
                                break
                            n += 1
                            yield

                njt = 4 * (sb + 1)
                segments = [(sb, h, 0, njt) for h in range(HPC)]
                if sb == NSBLK - 2:
                    segments += [(NSBLK - 1, 0, 0, SPLIT), (NSBLK - 1, 1, 0, SPLIT)]
                elif sb == NSBLK - 1:
                    segments = [
                        (sb, 0, SPLIT, njt),
                        (sb, 1, SPLIT, njt),
                        (sb, 2, 0, njt),
                        (sb, 3, 0, njt),
                    ]
                attn(segments, filler_iter_fn(seq), total)
            # ---- epilogue: output projection for the last block; exp is
            # done by now so ACT is free to take every drain, keeping DVE
            # clear for the last heads' normalization chain.  A few dummy
            # matmuls bridge the PE idle gap while that chain completes so
            # the clock gate stays at 8/8 for the final projection ----
            warm_ps = psc_pool.tile([1, 256], F32, name="scp")
            for _ in range(10):
                nc.tensor.matmul(
                    warm_ps[:], ones_bf[:], warm_sb[:],
                    start=True, stop=True, skip_group_check=True,
                )
            for _ in gen_outproj(NSBLK - 1, drain="act", fan_tail=True):
                pass

    nc.compile()
    return nc


def _rope_tables():
    inv_freq = 1.0 / (10000.0 ** (np.arange(0, D, 2, dtype=np.float64) / D))
    t = np.arange(S, dtype=np.float64)
    freqs = np.outer(t, inv_freq)  # (S, D/2)
    emb = np.concatenate([freqs, freqs], axis=-1)  # (S, D)
    cosT = np.cos(emb).T.astype(np.float32).copy()  # (D, S)
    sinT = np.sin(emb).T.astype(np.float32)
    sinTs = sinT.copy()
    sinTs[: D // 2] = -sinT[: D // 2]
    return cosT, np.ascontiguousarray(sinTs)


def _binmask():
    r = np.arange(128)[:, None]
    c = np.arange(SBLK)[None, :]
    blocks = [(r + o * 128 <= c).astype(np.float32) for o in range(4)]
    return np.ascontiguousarray(np.concatenate(blocks, axis=1))


def _bf16(a):
    return np.ascontiguousarray(np.asarray(a, dtype=ml_dtypes.bfloat16))


def _numpy_fallback(x, mask, wqkv, bqkv, wout, bout):
    qkv = x @ wqkv.T + bqkv
    q, k, v = np.split(qkv, 3, axis=-1)
    q = q.reshape(B, S, H, D).transpose(0, 2, 1, 3)
    k = k.reshape(B, S, H, D).transpose(0, 2, 1, 3)
    v = v.reshape(B, S, H, D).transpose(0, 2, 1, 3)
    inv_freq = 1.0 / (10000.0 ** (np.arange(0, D, 2, dtype=np.float32) / D))
    t = np.arange(S, dtype=np.float32)
    freqs = np.outer(t, inv_freq)
    emb = np.concatenate([freqs, freqs], axis=-1)
    cos, sin = np.cos(emb), np.sin(emb)

    def rot(a):
        a1, a2 = np.split(a, 2, axis=-1)
        return np.concatenate([-a2, a1], axis=-1)

    q = q * cos + rot(q) * sin
    k = k * cos + rot(k) * sin
    scores = np.einsum("bhqd,bhkd->bhqk", q, k) * SM_SCALE
    scores = np.where(mask, -np.inf, scores)
    scores = scores - scores.max(axis=-1, keepdims=True)
    w = np.exp(scores)
    w = w / w.sum(axis=-1, keepdims=True)
    ctx = np.einsum("bhqk,bhkd->bhqd", w, v)
    ctx = ctx.transpose(0, 2, 1, 3).reshape(B, S, E)
    return (ctx @ wout.T + bout).astype(np.float32)


def _pack_inputs(x, wqkv, wout):
    """Per-core host packing into the contiguous-DMA DRAM layouts."""
    cosT, sinTs = _rope_tables()  # (D, S)
    tabs = _bf16(np.concatenate([cosT, sinTs], axis=1))  # [128, 4096]
    bm = _bf16(_binmask())  # [128, 2048]

    in_maps = []
    for c in range(N_CORES):
        b, hg = divmod(c, GPB)
        cols = slice(hg * DPC, (hg + 1) * DPC)
        wq = wqkv[0 * E : 1 * E, :][cols, :]  # (512, E): [feat, E]
        wk = wqkv[1 * E : 2 * E, :][cols, :]
        wv = wqkv[2 * E : 3 * E, :][cols, :]
        wo = wout[:, cols]  # (E, 512)

        # x: [sb][qi][p][(e%4)*512+s] = x[b, sb*512+s, (4qi+e')*128+p]
        xb = np.asarray(x[b], dtype=ml_dtypes.bfloat16)  # (S, E)
        xr = xb.reshape(NSBLK, SBLK, NECH, 128)  # [sb, s, e, p]
        xqp = np.ascontiguousarray(
            xr.transpose(0, 2, 3, 1).reshape(NSBLK, 4, 4, 128, SBLK)
            .transpose(0, 1, 3, 2, 4).reshape(NSBLK, 4, 128, 2048)
        )
        # wqm: [m][p][e*128+c] = wq[m*128+c, e*128+p]
        def pack_w(w):  # w: (512, E) [feat, E]
            wr = w.reshape(HPC, 128, NECH, 128)  # [m, c, e, p]
            return np.ascontiguousarray(
                wr.transpose(0, 3, 2, 1).reshape(HPC, 128, 2048).astype(
                    ml_dtypes.bfloat16
                )
            )

        wqm = pack_w(wq)
        wkm = pack_w(wk)
        # wvh: [hh][p][(e%8)*512+f] = wv[f, e*128+p]
        wvr = wv.reshape(DPC, NECH, 128)  # [f, e, p]
        wvh = np.ascontiguousarray(
            wvr.transpose(1, 2, 0).reshape(2, 8, 128, DPC)
            .transpose(0, 2, 1, 3).reshape(2, 128, 4096).astype(ml_dtypes.bfloat16)
        )
        # woh: [hh][p][(h%2)*2048+cc] = wo[cc, h*128+p]
        wor = wo.reshape(E, HPC, 128)  # [cc, h, p]
        woh = np.ascontiguousarray(
            wor.transpose(1, 2, 0).reshape(2, 2, 128, E)
            .transpose(0, 2, 1, 3).reshape(2, 128, 4096).astype(ml_dtypes.bfloat16)
        )
        in_maps.append(
            {
                "xq": xqp,
                "wqm": wqm,
                "wkm": wkm,
                "wvh": wvh,
                "woh": woh,
                "tabs": tabs,
                "bmask": bm,
            }
        )
    return in_maps


def kernel(x, mask, wqkv, bqkv, wout, bout, **_):
    x = np.ascontiguousarray(np.asarray(x), dtype=np.float32)
    wqkv = np.ascontiguousarray(np.asarray(wqkv), dtype=np.float32)
    bqkv = np.asarray(bqkv, dtype=np.float32)
    wout = np.ascontiguousarray(np.asarray(wout), dtype=np.float32)
    bout = np.asarray(bout, dtype=np.float32)
    mask = np.asarray(mask)

    causal = np.array_equal(mask, np.triu(np.ones((S, S), dtype=bool), k=1))
    if not causal or np.any(bqkv):
        return _numpy_fallback(x, mask, wqkv, bqkv, wout, bout)

    if "nc" not in _CACHE:
        _CACHE["nc"] = _build_nc()
    nc = _CACHE["nc"]

    in_maps = _pack_inputs(x, wqkv, wout)

    res = bass_utils.run_bass_kernel_spmd(
        nc, in_maps, core_ids=list(range(N_CORES)), **_RUN_KWARGS
    )
    _CACHE["last_results"] = res

    out = np.empty((B, S, E), dtype=np.float32)
    for b in range(B):
        acc = res.results[b * GPB]["outT"].astype(np.float32)
        for g in range(1, GPB):
            acc += res.results[b * GPB + g]["outT"].astype(np.float32)
        # outT [p, m, s] -> [E=m*128+p, S] -> transpose to (S, E)
        out[b] = acc.transpose(1, 0, 2).reshape(E, S).T
    out += bout
    return out
